# revision 1
# baseline (speedup 1.0000x reference)
"""Trainium2 Bass kernel for nn_Hard_Extract_Weight_Sum_Cluster.

Data-parallel over 8 cores: 4 examples per core (batch dim of x, 48 rows of
atten). Per example the kernel:
  1. Streams atten (12 heads x [512,512]) and computes exact column sums via a
     fixed-point split: coarse = round(a to 2^-11 grid) (fp16-exact), fine =
     a - coarse (|fine| <= 2^-12, fp16). Both are column-summed with fp16
     ones-matmuls into f32 PSUM; the coarse sum is bit-exact regardless of
     accumulation order, so the ranking below reproduces the f64-exact order.
  2. Extracts head diagonals with strided DMAs, sums over heads (PE).
  3. Ranks tokens with an exact two-float comparator:
     cnt_less[k] = #{j: (hi_j - hi_k) < (lo_k - lo_j)} via fused
     scalar_tensor_tensor with accum_out.
  4. Builds head/tail masks from cnt_less, computes ordinal positions with
     triangular-matrix matmuls, softmax weights for the tail, and:
     - gathers the 251 head rows of x with indirect DMA (offsets computed
       on-device by counting prefix ranks),
     - pools the 260 tail rows into 5 clusters with a weighted [5x512] matmul.
"""

import numpy as np

import concourse.bacc as bacc
import concourse.bass as bass
import concourse.mybir as mybir
from concourse.bass_utils import run_bass_kernel_spmd
from concourse.tile import TileContext

f32 = mybir.dt.float32
f16 = mybir.dt.float16
i32 = mybir.dt.int32
Alu = mybir.AluOpType
ActFn = mybir.ActivationFunctionType

B, S, D, H = 32, 512, 768, 12
N_CORES = 8
EX = B // N_CORES          # 4 examples per core
P = 128
NC_CHUNK = S // P          # 4 row-blocks per head matrix
WIDE = S * NC_CHUNK        # 2048: one head = [128, 2048]
N_HEAD_OUT = 251           # CLS + 250 extracted tokens
# cnt_less thresholds (count of strictly-smaller among all 512 slots, CLS = -4)
#   head:  cnt >= 262   dropped: 261   tail: 1..260   CLS: 0


def build_nc():
    nc = bacc.Bacc()
    x_in = nc.declare_dram_parameter("x", [EX * S, D], f32, isOutput=False)
    at_in = nc.declare_dram_parameter("atten", [EX * H, S, S], f32, isOutput=False)
    ones_p_f16 = nc.declare_dram_parameter("c_ones_p_f16", [P, 1], f16, isOutput=False)
    ones_p_f32 = nc.declare_dram_parameter("c_ones_p_f32", [P, 1], f32, isOutput=False)
    ones_h = nc.declare_dram_parameter("c_ones_h", [H, 1], f32, isOutput=False)
    ones_r_f32 = nc.declare_dram_parameter("c_ones_r_f32", [1, P], f32, isOutput=False)
    ones_r_f16 = nc.declare_dram_parameter("c_ones_r_f16", [1, P], f16, isOutput=False)
    id4 = nc.declare_dram_parameter("c_id4", [4, 4], f32, isOutput=False)
    triu_sq = nc.declare_dram_parameter("c_triu", [P, P], f16, isOutput=False)
    ones_sq = nc.declare_dram_parameter("c_ones_sq", [P, P], f16, isOutput=False)
    tri_inc = nc.declare_dram_parameter("c_tri_inc", [P, NC_CHUNK * S], f16, isOutput=False)
    iota2 = nc.declare_dram_parameter("c_iota2", [P, 2], f32, isOutput=False)
    ones_1 = nc.declare_dram_parameter("c_ones_1", [1, 1], f32, isOutput=False)
    lowb = nc.declare_dram_parameter("c_lowb", [P, 5], f32, isOutput=False)
    highb = nc.declare_dram_parameter("c_highb", [P, 5], f32, isOutput=False)
    out = nc.declare_dram_parameter("out", [EX, 256, D], f32, isOutput=True)

    at_flat = at_in[:].rearrange("a b c -> (a b c)")

    with TileContext(nc) as tc:
        with tc.tile_pool(name="cst", bufs=1) as cst, \
             tc.tile_pool(name="big", bufs=4) as big, \
             tc.tile_pool(name="med", bufs=3) as med, \
             tc.tile_pool(name="sm", bufs=2) as sm, \
             tc.tile_pool(name="ps_acc", bufs=1, space="PSUM") as ps_acc, \
             tc.tile_pool(name="ps_big", bufs=2, space="PSUM") as ps_big, \
             tc.tile_pool(name="ps_sm", bufs=2, space="PSUM") as ps_sm:

            # ---- constants ----
            c_ones_p16 = cst.tile([P, 1], f16)
            nc.sync.dma_start(out=c_ones_p16, in_=ones_p_f16[:])
            c_ones_p32 = cst.tile([P, 1], f32)
            nc.sync.dma_start(out=c_ones_p32, in_=ones_p_f32[:])
            c_ones_h = cst.tile([H, 1], f32)
            nc.sync.dma_start(out=c_ones_h, in_=ones_h[:])
            c_ones_r32 = cst.tile([1, P], f32)
            nc.sync.dma_start(out=c_ones_r32, in_=ones_r_f32[:])
            c_ones_r16 = cst.tile([1, P], f16)
            nc.sync.dma_start(out=c_ones_r16, in_=ones_r_f16[:])
            c_id4 = cst.tile([4, 4], f32)
            nc.sync.dma_start(out=c_id4, in_=id4[:])
            c_triu = cst.tile([P, P], f16)
            nc.sync.dma_start(out=c_triu, in_=triu_sq[:])
            c_ones_sq = cst.tile([P, P], f16)
            nc.sync.dma_start(out=c_ones_sq, in_=ones_sq[:])
            c_tri = cst.tile([P, NC_CHUNK * S], f16)
            nc.sync.dma_start(out=c_tri, in_=tri_inc[:])
            c_iota2 = cst.tile([P, 2], f32)
            nc.sync.dma_start(out=c_iota2, in_=iota2[:])
            c_ones_1 = cst.tile([1, 1], f32)
            nc.sync.dma_start(out=c_ones_1, in_=ones_1[:])
            c_lowb = cst.tile([P, 5], f32)
            nc.sync.dma_start(out=c_lowb, in_=lowb[:])
            c_highb = cst.tile([P, 5], f32)
            nc.sync.dma_start(out=c_highb, in_=highb[:])

            # per-example accumulators in free layout (partition 0)
            hi_sb = [cst.tile([1, S], f32, name=f"hi_sb{b}") for b in range(EX)]
            lo_sb = [cst.tile([1, S], f32, name=f"lo_sb{b}") for b in range(EX)]
            dg_sb = [cst.tile([1, S], f32, name=f"dg_sb{b}") for b in range(EX)]

            # ================= stage A: stream + reduce =================
            for b in range(EX):
                hi_ps = ps_acc.tile([1, S], f32, tag="hi")
                lo_ps = ps_acc.tile([1, S], f32, tag="lo")
                diag_t = sm.tile([H, S], f32, tag="diag")
                for h in range(H):
                    base = (b * H + h) * S * S
                    nc.sync.dma_start(
                        out=diag_t[h:h + 1, :],
                        in_=at_flat[base:base + (S - 1) * (S + 1) + 1:S + 1]
                            .rearrange("(a b) -> a b", a=1))
                    a_t = big.tile([P, WIDE], f32, tag="a")
                    nc.sync.dma_start(
                        out=a_t.rearrange("p (k j) -> p k j", k=NC_CHUNK),
                        in_=at_in[b * H + h].rearrange("(k p) j -> p k j", p=P))
                    c_t = med.tile([P, WIDE], f16, tag="c")
                    nc.gpsimd.tensor_scalar(c_t, a_t, 4096.0, 4096.0,
                                            op0=Alu.add, op1=Alu.subtract)
                    f_t = med.tile([P, WIDE], f16, tag="f")
                    nc.vector.tensor_tensor(out=f_t, in0=a_t, in1=c_t,
                                            op=Alu.subtract)
                    for k in range(NC_CHUNK):
                        first = (h == 0 and k == 0)
                        last = (h == H - 1 and k == NC_CHUNK - 1)
                        nc.tensor.matmul(hi_ps, lhsT=c_ones_p16,
                                         rhs=c_t[:, k * S:(k + 1) * S],
                                         start=first, stop=last,
                                         skip_group_check=True)
                        nc.tensor.matmul(lo_ps, lhsT=c_ones_p16,
                                         rhs=f_t[:, k * S:(k + 1) * S],
                                         start=first, stop=last,
                                         skip_group_check=True)
                nc.scalar.copy(hi_sb[b], hi_ps)
                nc.scalar.copy(lo_sb[b], lo_ps)
                dg_ps = ps_sm.tile([1, S], f32, tag="scr")
                nc.tensor.matmul(dg_ps, lhsT=c_ones_h, rhs=diag_t,
                                 start=True, stop=True)
                nc.scalar.copy(dg_sb[b], dg_ps)

            # ================= finalize hi/lo =================
            lo1_sb = [cst.tile([1, S], f32, name=f"lo1_sb{b}") for b in range(EX)]
            for b in range(EX):
                nc.vector.tensor_tensor(out=lo1_sb[b], in0=lo_sb[b],
                                        in1=dg_sb[b], op=Alu.subtract)
                nc.vector.memset(hi_sb[b][:, 0:1], -4.0)
                nc.vector.memset(lo1_sb[b][:, 0:1], 0.0)

            # transposes via ones[1,1] matmul: [1,128] slice -> [128,1] col 4c+b
            hiT_ps = ps_sm.tile([P, 4 * EX], f32, tag="scr")
            loT_ps = ps_sm.tile([P, 4 * EX], f32, tag="scr2")
            for c in range(NC_CHUNK):
                for b in range(EX):
                    col = 4 * c + b
                    nc.tensor.matmul(hiT_ps[:, col:col + 1],
                                     lhsT=hi_sb[b][0:1, c * P:(c + 1) * P],
                                     rhs=c_ones_1, start=True, stop=True)
                    nc.tensor.matmul(loT_ps[:, col:col + 1],
                                     lhsT=lo1_sb[b][0:1, c * P:(c + 1) * P],
                                     rhs=c_ones_1, start=True, stop=True)
            hiT = cst.tile([P, 4 * EX], f32)
            nc.scalar.copy(hiT, hiT_ps)
            loT = cst.tile([P, 4 * EX], f32)
            nc.scalar.copy(loT, loT_ps)

            # softmax numerators (tail weights), all examples at once
            s_t = cst.tile([P, 4 * EX], f32)
            nc.vector.tensor_tensor(out=s_t, in0=hiT, in1=loT, op=Alu.add)
            e_t = cst.tile([P, 4 * EX], f32)
            bias_t = cst.tile([P, 1], f32)
            nc.vector.memset(bias_t, -256.0)
            nc.scalar.activation(e_t, s_t, ActFn.Exp, bias=bias_t[:, 0:1],
                                 scale=1.0 / 12.0)

            cnt = cst.tile([P, 4 * EX], f32)
            m_ext = cst.tile([P, 4 * EX], f16)
            m_tail = cst.tile([P, 4 * EX], f16)
            e_m = cst.tile([P, 4 * EX], f32)

            for b in range(EX):
                # broadcast hi/lo rows across partitions
                bch_ps = ps_big.tile([P, S], f32, tag="bc")
                nc.tensor.matmul(bch_ps, lhsT=c_ones_r32,
                                 rhs=hi_sb[b], start=True, stop=True)
                bch = med.tile([P, S], f32, tag="bch")
                nc.scalar.copy(bch, bch_ps)
                bcl_ps = ps_big.tile([P, S], f32, tag="bc")
                nc.tensor.matmul(bcl_ps, lhsT=c_ones_r32,
                                 rhs=lo1_sb[b], start=True, stop=True)
                bcl = med.tile([P, S], f32, tag="bcl")
                nc.scalar.copy(bcl, bcl_ps)

                # exact two-float rank: cnt_less[k] = sum_j (v_j < v_k)
                for c in range(NC_CHUNK):
                    col = 4 * c + b
                    F_t = sm.tile([P, S], f32, tag="F")
                    nc.gpsimd.tensor_scalar(F_t, bcl, -1.0, loT[:, col:col + 1],
                                            op0=Alu.mult, op1=Alu.add)
                    scr_t = sm.tile([P, S], f16, tag="scr")
                    nc.vector.scalar_tensor_tensor(
                        out=scr_t, in0=bch, scalar=hiT[:, col:col + 1], in1=F_t,
                        op0=Alu.subtract, op1=Alu.is_lt,
                        accum_out=cnt[:, col:col + 1])

            # masks from cnt_less
            nc.vector.tensor_scalar(m_ext, cnt, 261.5, None, op0=Alu.is_ge)
            mta = sm.tile([P, 4 * EX], f16, tag="mta")
            nc.vector.tensor_scalar(mta, cnt, 0.5, None, op0=Alu.is_gt)
            mtb = sm.tile([P, 4 * EX], f16, tag="mtb")
            nc.vector.tensor_scalar(mtb, cnt, 260.5, None, op0=Alu.is_lt)
            nc.vector.tensor_tensor(out=m_tail, in0=mta, in1=mtb, op=Alu.mult)
            # CLS (k=0, chunk 0, partition 0) joins the extract set
            nc.vector.memset(m_ext[0:1, 0:EX], 1.0)
            nc.vector.tensor_tensor(out=e_m, in0=e_t, in1=m_tail, op=Alu.mult)

            # tail normalization: Z per example, then 1/(53 Z) per partition
            z_ps = ps_sm.tile([1, 4 * EX], f32, tag="scr")
            nc.tensor.matmul(z_ps, lhsT=c_ones_p32, rhs=e_m, start=True, stop=True)
            z_sb = sm.tile([1, 4 * EX], f32, tag="zsb")
            nc.scalar.copy(z_sb, z_ps)
            z4 = sm.tile([1, EX], f32, tag="z4")
            nc.vector.tensor_reduce(
                z4, z_sb.rearrange("a (c b) -> a b c", b=EX),
                axis=mybir.AxisListType.X, op=Alu.add)
            rz4 = sm.tile([1, EX], f32, tag="rz4")
            nc.vector.reciprocal(rz4, z4)

            for b in range(EX):
                # P_ext inclusive prefix (free layout) -> src offsets
                pe_ps = ps_sm.tile([1, S], f32, tag="scr")
                for c in range(NC_CHUNK):
                    nc.tensor.matmul(pe_ps, lhsT=m_ext[:, 4 * c + b:4 * c + b + 1],
                                                     rhs=c_tri[:, c * S:(c + 1) * S],
                                     start=(c == 0),
                                     stop=(c == NC_CHUNK - 1),
                                     skip_group_check=True)
                pe_sb = sm.tile([1, S], f16, tag="pesb")
                nc.scalar.copy(pe_sb, pe_ps)
                bcp_ps = ps_big.tile([P, S], f32, tag="bc")
                nc.tensor.matmul(bcp_ps, lhsT=c_ones_r16, rhs=pe_sb,
                                 start=True, stop=True)
                bcp_sb = med.tile([P, S], f32, tag="bcpsb")
                nc.scalar.copy(bcp_sb, bcp_ps)
                src_f = sm.tile([P, 2], f32, tag="srcf")
                for rc in range(2):
                    scr2 = sm.tile([P, S], f16, tag="scr2")
                    nc.vector.scalar_tensor_tensor(
                        out=scr2, in0=bcp_sb, scalar=c_iota2[:, rc:rc + 1],
                        in1=bcp_sb, op0=Alu.is_le, op1=Alu.bypass,
                        accum_out=src_f[:, rc:rc + 1])
                src_i = sm.tile([P, 2], i32, tag="srci")
                nc.vector.tensor_scalar(src_i, src_f, float(b * S), None,
                                        op0=Alu.add)
                # gather head rows of x -> out rows 0..250
                g0 = med.tile([P, D], f32, tag="g0")
                nc.gpsimd.indirect_dma_start(
                    out=g0, out_offset=None, in_=x_in[:],
                    in_offset=bass.IndirectOffsetOnAxis(ap=src_i[:, 0:1], axis=0))
                nc.sync.dma_start(out=out[b, 0:P, :], in_=g0)
                g1 = med.tile([P, D], f32, tag="g1")
                nc.gpsimd.indirect_dma_start(
                    out=g1[0:N_HEAD_OUT - P, :], out_offset=None, in_=x_in[:],
                    in_offset=bass.IndirectOffsetOnAxis(ap=src_i[0:N_HEAD_OUT - P, 1:2],
                                                        axis=0))
                nc.sync.dma_start(out=out[b, P:N_HEAD_OUT, :],
                                  in_=g1[0:N_HEAD_OUT - P, :])

                # tail positions in partition layout (inclusive prefix)
                tp_ps = ps_sm.tile([P, NC_CHUNK], f32, tag="scr")
                for c in range(NC_CHUNK):
                    for cc in range(c + 1):
                        nc.tensor.matmul(
                            tp_ps[:, c:c + 1],
                            lhsT=(c_triu if cc == c else c_ones_sq),
                            rhs=m_tail[:, 4 * cc + b:4 * cc + b + 1],
                            start=(cc == 0), stop=(cc == c),
                            skip_group_check=True)
                tp_sb = sm.tile([P, NC_CHUNK], f32, tag="tpsb")
                nc.scalar.copy(tp_sb, tp_ps)

                # 1/(53 Z_b) broadcast across partitions
                rzb_ps = ps_sm.tile([P, 1], f32, tag="scr2")
                nc.tensor.matmul(rzb_ps, lhsT=c_ones_r32, rhs=rz4[0:1, b:b + 1],
                                 start=True, stop=True)
                rz53 = sm.tile([P, 1], f32, tag="rz53")
                nc.vector.tensor_scalar(rz53, rzb_ps, 1.0 / 53.0, None,
                                        op0=Alu.mult)

                # weighted cluster matmul
                x_t = big.tile([P, NC_CHUNK * D], f32, tag="x")
                nc.sync.dma_start(
                    out=x_t.rearrange("p (k d) -> p k d", k=NC_CHUNK),
                    in_=x_in[b * S:(b + 1) * S, :].rearrange("(k p) d -> p k d", p=P))
                cl_a = ps_sm.tile([5, S], f32, tag="scr")
                cl_b = ps_sm.tile([5, D - S], f32, tag="scr2")
                for c in range(NC_CHUNK):
                    o2 = sm.tile([P, 5], f32, tag="o2")
                    nc.vector.tensor_scalar(o2, c_highb, tp_sb[:, c:c + 1], None,
                                            op0=Alu.is_gt)
                    oh = sm.tile([P, 5], f32, tag="oh")
                    nc.vector.scalar_tensor_tensor(
                        out=oh, in0=c_lowb, scalar=tp_sb[:, c:c + 1], in1=o2,
                        op0=Alu.is_lt, op1=Alu.mult)
                    wq = sm.tile([P, 5], f32, tag="wq")
                    nc.vector.tensor_scalar(
                        wq, oh, e_m[:, 4 * c + b:4 * c + b + 1], rz53[:, 0:1],
                        op0=Alu.mult, op1=Alu.mult)
                    nc.tensor.matmul(cl_a, lhsT=wq, rhs=x_t[:, c * D:c * D + S],
                                     start=(c == 0), stop=(c == NC_CHUNK - 1),
                                     skip_group_check=True)
                    nc.tensor.matmul(cl_b, lhsT=wq, rhs=x_t[:, c * D + S:(c + 1) * D],
                                     start=(c == 0), stop=(c == NC_CHUNK - 1),
                                     skip_group_check=True)
                cl_sb = sm.tile([5, D], f32, tag="clsb")
                nc.scalar.copy(cl_sb[:, 0:S], cl_a)
                nc.scalar.copy(cl_sb[:, S:D], cl_b)
                nc.sync.dma_start(out=out[b, N_HEAD_OUT:256, :], in_=cl_sb)

    nc.compile()
    return nc


_NC_CACHE = {}


def _consts():
    tri = np.zeros((P, NC_CHUNK * S), np.float16)
    for c in range(NC_CHUNK):
        for p in range(P):
            tri[p, c * S + c * P + p:(c + 1) * S] = 1.0
    iota2 = (np.arange(P, dtype=np.float32)[:, None]
             + np.array([0.0, 128.0], np.float32)[None, :])
    lowb = np.tile((53.0 * np.arange(5, dtype=np.float32) + 0.5)[None, :], (P, 1))
    highb = np.tile((53.0 * np.arange(5, dtype=np.float32) + 53.5)[None, :], (P, 1))
    return {
        "c_ones_p_f16": np.ones((P, 1), np.float16),
        "c_ones_p_f32": np.ones((P, 1), np.float32),
        "c_ones_h": np.ones((H, 1), np.float32),
        "c_ones_r_f32": np.ones((1, P), np.float32),
        "c_ones_r_f16": np.ones((1, P), np.float16),
        "c_id4": np.eye(4, dtype=np.float32),
        "c_triu": np.triu(np.ones((P, P))).astype(np.float16),
        "c_ones_sq": np.ones((P, P), np.float16),
        "c_tri_inc": tri,
        "c_iota2": iota2,
        "c_ones_1": np.ones((1, 1), np.float32),
        "c_lowb": lowb,
        "c_highb": highb,
    }


def kernel(x: np.ndarray, atten: np.ndarray, trace: bool = False):
    if "nc" not in _NC_CACHE:
        _NC_CACHE["nc"] = build_nc()
    nc = _NC_CACHE["nc"]
    x = np.ascontiguousarray(np.asarray(x, np.float32))
    atten = np.ascontiguousarray(np.asarray(atten, np.float32))
    consts = _consts()
    in_maps = []
    for ci in range(N_CORES):
        in_maps.append({
            "x": x[ci * EX:(ci + 1) * EX].reshape(EX * S, D),
            "atten": atten[ci * EX * H:(ci + 1) * EX * H],
            **consts,
        })
    res = run_bass_kernel_spmd(nc, in_maps, list(range(N_CORES)), trace=trace)
    _NC_CACHE["last_res"] = res
    out = np.concatenate([res.results[ci]["out"] for ci in range(N_CORES)], axis=0)
    if trace:
        return out, res
    return out



# revision 53
# speedup vs baseline: 1.0213x; 1.0213x over previous
"""Trainium2 Bass kernel for nn_Hard_Extract_Weight_Sum_Cluster.

Data-parallel over 8 cores: 4 examples per core (batch dim of x, 48 rows of
atten). Per example the kernel:
  1. Streams atten (12 heads x [512,512]) and computes exact column sums via a
     fixed-point split: coarse = round(a to 2^-11 grid) (fp16-exact), fine =
     a - coarse (|fine| <= 2^-12, fp16). Both are column-summed with fp16
     ones-matmuls into f32 PSUM; the coarse sum is bit-exact regardless of
     accumulation order, so the ranking below reproduces the f64-exact order.
  2. Loads all 48 head diagonals with strided DMAs ([12,512] each), sums
     them per example with a 12-partition ones-matmul, subtracts from the
     fine plane (lo1 = lo - dg).
  3. Ranks tokens with an exact two-float comparator:
     cnt_less[k] = #{j: (hi_j - hi_k) < (lo_k - lo_j)} via fused
     scalar_tensor_tensor with accum_out.
  4. Builds head/tail masks from cnt_less, computes ordinal positions with
     triangular-matrix matmuls, softmax weights for the tail, and:
     - gathers the 251 head rows of x with indirect DMA (offsets computed
       on-device by counting prefix ranks),
     - pools the 260 tail rows into 5 clusters with a weighted bf16 matmul.

Schedule: the 1MB atten head-loads round-robin across the SP and Activation
DMA queues (different queues overlap; same-queue DMAs serialize); the
coarse-plane rounding runs on Pool, the fine-plane subtract is split
Pool/DVE by columns. Finalize work for example b is cut into five pieces
emitted between the streaming heads of example b+1 so every engine stays
fed across example boundaries. Consumers read PSUM directly where possible
to skip PSUM->SBUF copies.
"""

import numpy as np

import concourse.bacc as bacc
import concourse.bass as bass
import concourse.mybir as mybir
from concourse.bass_utils import run_bass_kernel_spmd
from concourse.tile import TileContext

f32 = mybir.dt.float32
f16 = mybir.dt.float16
bf16 = mybir.dt.bfloat16
i32 = mybir.dt.int32
Alu = mybir.AluOpType
ActFn = mybir.ActivationFunctionType

B, S, D, H = 32, 512, 768, 12
N_CORES = 8
EX = B // N_CORES          # 4 examples per core
P = 128
NC_CHUNK = S // P          # 4 row-blocks per head matrix
WIDE = S * NC_CHUNK        # 2048: one head = [128, 2048]
N_HEAD_OUT = 251           # CLS + 250 extracted tokens
FSPL = 256                 # leading fine-plane columns computed on Pool
# cnt_less thresholds (count of strictly-smaller among all 512 slots, CLS = -4)
#   head:  cnt >= 262   dropped: 261   tail: 1..260   CLS: 0


def build_nc():
    nc = bacc.Bacc()
    x_in = nc.declare_dram_parameter("x", [EX * S, D], f32, isOutput=False)
    at_in = nc.declare_dram_parameter("atten", [EX * H, S, S], f32, isOutput=False)
    ones_p_f16 = nc.declare_dram_parameter("c_ones_p_f16", [P, 1], f16, isOutput=False)
    ones_p_f32 = nc.declare_dram_parameter("c_ones_p_f32", [P, 1], f32, isOutput=False)
    ones_r_f32 = nc.declare_dram_parameter("c_ones_r_f32", [1, P], f32, isOutput=False)
    ones_r_f16 = nc.declare_dram_parameter("c_ones_r_f16", [1, P], f16, isOutput=False)
    triu_sq = nc.declare_dram_parameter("c_triu", [P, P], f16, isOutput=False)
    ones_sq = nc.declare_dram_parameter("c_ones_sq", [P, P], f16, isOutput=False)
    tri_inc = nc.declare_dram_parameter("c_tri_inc", [P, NC_CHUNK * S], f16, isOutput=False)
    iota2 = nc.declare_dram_parameter("c_iota2", [P, 2], f32, isOutput=False)
    ones_1 = nc.declare_dram_parameter("c_ones_1", [1, 1], f32, isOutput=False)
    lowb = nc.declare_dram_parameter("c_lowb", [P, 5], f32, isOutput=False)
    highb = nc.declare_dram_parameter("c_highb", [P, 5], f32, isOutput=False)
    out = nc.declare_dram_parameter("out", [EX, 256, D], f32, isOutput=True)

    # strided view of all 48 head diagonals: [48, 512] with column step S+1
    at_diag = at_in[:].rearrange("a b c -> a (b c)")[:, 0:(S - 1) * (S + 1) + 1:S + 1]

    with TileContext(nc) as tc:
        with tc.tile_pool(name="cst", bufs=1) as cst, \
             tc.tile_pool(name="big", bufs=5) as big, \
             tc.tile_pool(name="xp", bufs=2) as xp, \
             tc.tile_pool(name="med", bufs=4) as med, \
             tc.tile_pool(name="fin", bufs=2) as fin, \
             tc.tile_pool(name="sm", bufs=2) as sm, \
             tc.tile_pool(name="ps_acc", bufs=1, space="PSUM") as ps_acc, \
             tc.tile_pool(name="ps_big", bufs=2, space="PSUM") as ps_big, \
             tc.tile_pool(name="ps_sm", bufs=1, space="PSUM") as ps_sm:

            # ---- constants ----
            c_ones_p16 = cst.tile([P, 1], f16)
            nc.sync.dma_start(out=c_ones_p16, in_=ones_p_f16[:])
            c_ones_p32 = cst.tile([P, 1], f32)
            nc.sync.dma_start(out=c_ones_p32, in_=ones_p_f32[:])
            c_ones_r32 = cst.tile([1, P], f32)
            nc.sync.dma_start(out=c_ones_r32, in_=ones_r_f32[:])
            c_ones_r16 = cst.tile([1, P], f16)
            nc.sync.dma_start(out=c_ones_r16, in_=ones_r_f16[:])
            c_triu = cst.tile([P, P], f16)
            nc.gpsimd.dma_start(out=c_triu, in_=triu_sq[:])
            c_ones_sq = cst.tile([P, P], f16)
            nc.gpsimd.dma_start(out=c_ones_sq, in_=ones_sq[:])
            c_tri = cst.tile([P, NC_CHUNK * S], f16)
            nc.gpsimd.dma_start(out=c_tri, in_=tri_inc[:])
            c_iota2 = cst.tile([P, 2], f32)
            nc.gpsimd.dma_start(out=c_iota2, in_=iota2[:])
            c_ones_1 = cst.tile([1, 1], f32)
            nc.gpsimd.dma_start(out=c_ones_1, in_=ones_1[:])
            c_lowb = cst.tile([P, 5], f32)
            nc.gpsimd.dma_start(out=c_lowb, in_=lowb[:])
            c_highb = cst.tile([P, 5], f32)
            nc.gpsimd.dma_start(out=c_highb, in_=highb[:])

            # head diagonals, one strided DMA per example, emitted lazily
            diag_t = cst.tile([H, EX * S], f32)

            def emit_diag_dma(b):
                nc.scalar.dma_start(
                    out=diag_t[:, b * S:(b + 1) * S],
                    in_=at_diag[b * H:(b + 1) * H, :])

            # per-example accumulators in free layout (partition 0)
            hi_sb = [cst.tile([1, S], f32, name=f"hi_sb{b}") for b in range(EX)]
            lo1_sb = [cst.tile([1, S], f32, name=f"lo1_sb{b}") for b in range(EX)]

            # batched finalize tiles (columns 4b+c)
            hiT = cst.tile([P, 4 * EX], f32)
            loT = cst.tile([P, 4 * EX], f32)
            e_t = cst.tile([P, 4 * EX], f32)
            cnt = cst.tile([P, 4 * EX], f32)
            m_ext = cst.tile([P, 4 * EX], f16)
            m_tail = cst.tile([P, 4 * EX], f16)
            e_m = cst.tile([P, 4 * EX], f32)
            bias_t = cst.tile([P, 1], f32)
            nc.vector.memset(bias_t, -256.0)

            # queue schedule for the 48 head loads: SP 28, Act 20. Act slots
            # sit on even heads so each finalize piece (fired after odd
            # heads) has at most one Act DMA queued ahead of its copies.
            _qcycle = "assassasassassasasaassas"

            def a_queue(i):
                ch = _qcycle[i % len(_qcycle)]
                return {"s": nc.sync, "a": nc.scalar, "p": nc.gpsimd}[ch]

            def emit_stage_a(b, pieces, x_load):
                """Stream example b; emit pending finalize `pieces` (from
                example b-1) between heads."""
                hi_ps = ps_acc.tile([1, S], f32, tag="hi")
                lo_ps = ps_acc.tile([1, S], f32, tag="lo")
                # fire the pending finalize pieces early in the stream
                fire = dict(zip((0, 1, 2, 3, 5), pieces))
                for h in range(H):
                    if h == 6:
                        x_load()
                    a_t = big.tile([P, WIDE], f32, tag="a")
                    a_queue(b * H + h).dma_start(
                        out=a_t.rearrange("p (k j) -> p k j", k=NC_CHUNK),
                        in_=at_in[b * H + h].rearrange("(k p) j -> p k j", p=P))
                    c_t = med.tile([P, WIDE], f16, tag="c")
                    nc.gpsimd.tensor_scalar(c_t[:, FSPL:], a_t[:, FSPL:],
                                            4096.0, 4096.0,
                                            op0=Alu.add, op1=Alu.subtract)
                    f_t = med.tile([P, WIDE], f16, tag="f")
                    nc.vector.tensor_tensor(out=f_t[:, FSPL:],
                                            in0=a_t[:, FSPL:],
                                            in1=c_t[:, FSPL:],
                                            op=Alu.subtract)
                    nc.gpsimd.tensor_scalar(c_t[:, 0:FSPL], a_t[:, 0:FSPL],
                                            4096.0, 4096.0,
                                            op0=Alu.add, op1=Alu.subtract)
                    nc.gpsimd.tensor_tensor(out=f_t[:, 0:FSPL],
                                            in0=a_t[:, 0:FSPL],
                                            in1=c_t[:, 0:FSPL],
                                            op=Alu.subtract)
                    for k in range(NC_CHUNK):
                        first = (h == 0 and k == 0)
                        last = (h == H - 1 and k == NC_CHUNK - 1)
                        nc.tensor.matmul(hi_ps, lhsT=c_ones_p16,
                                         rhs=c_t[:, k * S:(k + 1) * S],
                                         start=first, stop=last,
                                         skip_group_check=True)
                        nc.tensor.matmul(lo_ps, lhsT=c_ones_p16,
                                         rhs=f_t[:, k * S:(k + 1) * S],
                                         start=first, stop=last,
                                         skip_group_check=True)
                    if h in fire:
                        fire[h]()
                return hi_ps, lo_ps

            def finalize_pieces(b, hi_ps, lo_ps, x_t):
                """Return the five finalize pieces for example b as closures."""
                st = {}

                def p1():
                    # bf16 copy of x for the cluster matmul (off the tail path)
                    x_bf = xp.tile([P, NC_CHUNK * D], bf16, tag="xbf")
                    st["x_bf"] = x_bf
                    nc.scalar.copy(x_bf, x_t)
                    # head-diagonal sum, hi/lo fixup, transposes
                    dg_ps = ps_sm.tile([1, S], f32, tag="dg")
                    nc.tensor.matmul(dg_ps, lhsT=c_ones_p32[0:H, 0:1],
                                     rhs=diag_t[0:H, b * S:(b + 1) * S],
                                     start=True, stop=True)
                    nc.scalar.copy(hi_sb[b], hi_ps)
                    # only one non-scalar PSUM operand allowed per instruction
                    dg_sb = sm.tile([1, S], f32, tag="dgsb")
                    nc.scalar.copy(dg_sb, dg_ps)
                    nc.vector.tensor_tensor(out=lo1_sb[b], in0=lo_ps,
                                            in1=dg_sb, op=Alu.subtract)
                    nc.vector.memset(hi_sb[b][:, 0:1], -4.0)
                    nc.vector.memset(lo1_sb[b][:, 0:1], 0.0)
                    hiT_ps = ps_sm.tile([P, NC_CHUNK], f32, tag="scr")
                    loT_ps = ps_sm.tile([P, NC_CHUNK], f32, tag="scr2")
                    for c in range(NC_CHUNK):
                        nc.tensor.matmul(hiT_ps[:, c:c + 1],
                                         lhsT=hi_sb[b][0:1, c * P:(c + 1) * P],
                                         rhs=c_ones_1, start=True, stop=True)
                        nc.tensor.matmul(loT_ps[:, c:c + 1],
                                         lhsT=lo1_sb[b][0:1, c * P:(c + 1) * P],
                                         rhs=c_ones_1, start=True, stop=True)
                    st["hiT"], st["loT"] = hiT_ps, loT_ps
                    cols = slice(4 * b, 4 * b + NC_CHUNK)
                    nc.scalar.copy(loT[:, cols], loT_ps)

                def p2():
                    cols = slice(4 * b, 4 * b + NC_CHUNK)
                    s_t = sm.tile([P, NC_CHUNK], f32, tag="st")
                    nc.vector.tensor_tensor(out=s_t, in0=st["hiT"],
                                            in1=loT[:, cols], op=Alu.add)
                    nc.scalar.activation(e_t[:, cols], s_t, ActFn.Exp,
                                         bias=bias_t[:, 0:1], scale=1.0 / 12.0)
                    # broadcasts land in SBUF: Pool (GPSIMD) cannot read PSUM
                    bch_ps = ps_big.tile([P, S], f32, tag="bc")
                    nc.tensor.matmul(bch_ps, lhsT=c_ones_r32,
                                     rhs=hi_sb[b], start=True, stop=True)
                    bcl_ps = ps_big.tile([P, S], f32, tag="bc")
                    nc.tensor.matmul(bcl_ps, lhsT=c_ones_r32,
                                     rhs=lo1_sb[b], start=True, stop=True)
                    bcl_sb = fin.tile([P, S], f32, tag="bcl")
                    nc.scalar.copy(bcl_sb, bcl_ps)
                    st["bch"], st["bcl"] = bch_ps, bcl_sb

                def p3():
                    # exact two-float rank, masks, softmax numerators;
                    # F/scr split across Pool and DVE so the chunk chains
                    # run in parallel
                    cols = slice(4 * b, 4 * b + NC_CHUNK)
                    F_ts = []
                    for c in range(NC_CHUNK):
                        F_t = sm.tile([P, S], f32, tag=f"F{c}")
                        nc.gpsimd.tensor_scalar(F_t, st["bcl"], -1.0,
                                                loT[:, 4 * b + c:4 * b + c + 1],
                                                op0=Alu.mult, op1=Alu.add)
                        F_ts.append(F_t)
                    for c in range(NC_CHUNK):
                        col = 4 * b + c
                        scr_t = sm.tile([P, S], f16, tag="scr")
                        nc.vector.scalar_tensor_tensor(
                            out=scr_t, in0=st["bch"],
                            scalar=st["hiT"][:, c:c + 1],
                            in1=F_ts[c], op0=Alu.subtract, op1=Alu.is_lt,
                            accum_out=cnt[:, col:col + 1])
                    nc.vector.tensor_scalar(m_ext[:, cols], cnt[:, cols], 261.5,
                                            None, op0=Alu.is_ge)
                    mta = sm.tile([P, NC_CHUNK], f16, tag="mta")
                    nc.vector.tensor_scalar(mta, cnt[:, cols], 0.5, None,
                                            op0=Alu.is_gt)
                    mtb = sm.tile([P, NC_CHUNK], f16, tag="mtb")
                    nc.vector.tensor_scalar(mtb, cnt[:, cols], 260.5, None,
                                            op0=Alu.is_lt)
                    nc.vector.tensor_tensor(out=m_tail[:, cols], in0=mta,
                                            in1=mtb, op=Alu.mult)
                    nc.vector.memset(m_ext[0:1, 4 * b:4 * b + 1], 1.0)
                    nc.vector.tensor_tensor(out=e_m[:, cols], in0=e_t[:, cols],
                                            in1=m_tail[:, cols], op=Alu.mult)

                def p4():
                    # tail ordinal positions + normalization (PE work first so
                    # it overlaps the DVE src chain below)
                    cols = slice(4 * b, 4 * b + NC_CHUNK)
                    z_ps = ps_sm.tile([1, NC_CHUNK], f32, tag="scr")
                    nc.tensor.matmul(z_ps, lhsT=c_ones_p32, rhs=e_m[:, cols],
                                     start=True, stop=True)
                    tp_ps = ps_sm.tile([P, NC_CHUNK], f32, tag="tp")
                    for c in range(NC_CHUNK):
                        for cc in range(c + 1):
                            nc.tensor.matmul(
                                tp_ps[:, c:c + 1],
                                lhsT=(c_triu if cc == c else c_ones_sq),
                                rhs=m_tail[:, 4 * b + cc:4 * b + cc + 1],
                                start=(cc == 0), stop=(cc == c),
                                skip_group_check=True)
                    st["tp"] = tp_ps
                    z1 = sm.tile([1, 1], f32, tag="z1")
                    nc.vector.tensor_reduce(z1, z_ps, axis=mybir.AxisListType.X,
                                            op=Alu.add)
                    rz1 = sm.tile([1, 1], f32, tag="rz1")
                    nc.vector.reciprocal(rz1, z1)
                    rzb_ps = ps_sm.tile([P, 1], f32, tag="scr2")
                    nc.tensor.matmul(rzb_ps, lhsT=c_ones_r32, rhs=rz1,
                                     start=True, stop=True)
                    rz53 = sm.tile([P, 1], f32, tag="rz53")
                    nc.vector.tensor_scalar(rz53, rzb_ps, 1.0 / 53.0, None,
                                            op0=Alu.mult)
                    st["rz53"] = rz53
                    pe_ps = ps_sm.tile([1, S], f32, tag="scr")
                    for c in range(NC_CHUNK):
                        nc.tensor.matmul(pe_ps,
                                         lhsT=m_ext[:, 4 * b + c:4 * b + c + 1],
                                         rhs=c_tri[:, c * S:(c + 1) * S],
                                         start=(c == 0),
                                         stop=(c == NC_CHUNK - 1),
                                         skip_group_check=True)
                    pe_sb = sm.tile([1, S], f16, tag="pesb")
                    nc.scalar.copy(pe_sb, pe_ps)
                    bcp_ps = ps_big.tile([P, S], f32, tag="bc")
                    nc.tensor.matmul(bcp_ps, lhsT=c_ones_r16, rhs=pe_sb,
                                     start=True, stop=True)
                    bcp_sb = fin.tile([P, S], f32, tag="bcpsb")
                    nc.scalar.copy(bcp_sb, bcp_ps)
                    src_f = sm.tile([P, 2], f32, tag="srcf")
                    for rc in range(2):
                        scr2 = sm.tile([P, S], f16, tag="scr2")
                        nc.vector.scalar_tensor_tensor(
                            out=scr2, in0=bcp_sb, scalar=c_iota2[:, rc:rc + 1],
                            in1=bcp_sb, op0=Alu.is_le, op1=Alu.bypass,
                            accum_out=src_f[:, rc:rc + 1])
                    # clamp the 5 unused trailing offsets of column 1 in
                    # bounds (they land on x row EX*S-1 and are never read)
                    src_i = sm.tile([P, 2], i32, tag="srci")
                    nc.vector.tensor_scalar(src_i, src_f, float(b * S),
                                            float(EX * S - 1),
                                            op0=Alu.add, op1=Alu.min)
                    g0 = fin.tile([P, D], f32, tag="g0")
                    nc.gpsimd.indirect_dma_start(
                        out=g0, out_offset=None, in_=x_in[:],
                        in_offset=bass.IndirectOffsetOnAxis(ap=src_i[:, 0:1],
                                                            axis=0))
                    nc.scalar.dma_start(out=out[b, 0:P, :], in_=g0)
                    g1 = fin.tile([P, D], f32, tag="g1")
                    nc.gpsimd.indirect_dma_start(
                        out=g1[0:N_HEAD_OUT - P, :], out_offset=None,
                        in_=x_in[:],
                        in_offset=bass.IndirectOffsetOnAxis(
                            ap=src_i[0:N_HEAD_OUT - P, 1:2], axis=0))
                    nc.sync.dma_start(out=out[b, P:N_HEAD_OUT, :],
                                      in_=g1[0:N_HEAD_OUT - P, :])

                def p5():
                    # tail weights + weighted cluster matmul
                    tp_ps = st["tp"]
                    rz53 = st["rz53"]
                    x_bf = st["x_bf"]
                    cl_a = ps_sm.tile([5, S], f32, tag="scr")
                    cl_b = ps_sm.tile([5, D - S], f32, tag="scr2")
                    for c in range(NC_CHUNK):
                        o2 = sm.tile([P, 5], f32, tag="o2")
                        nc.vector.tensor_scalar(o2, c_highb, tp_ps[:, c:c + 1],
                                                None, op0=Alu.is_gt)
                        oh = sm.tile([P, 5], f32, tag="oh")
                        nc.vector.scalar_tensor_tensor(
                            out=oh, in0=c_lowb, scalar=tp_ps[:, c:c + 1],
                            in1=o2, op0=Alu.is_lt, op1=Alu.mult)
                        wq = sm.tile([P, 5], bf16, tag="wq")
                        nc.vector.tensor_scalar(
                            wq, oh, e_m[:, 4 * b + c:4 * b + c + 1],
                            rz53[:, 0:1], op0=Alu.mult, op1=Alu.mult)
                        nc.tensor.matmul(cl_a, lhsT=wq,
                                         rhs=x_bf[:, c * D:c * D + S],
                                         start=(c == 0),
                                         stop=(c == NC_CHUNK - 1),
                                         skip_group_check=True)
                        nc.tensor.matmul(cl_b, lhsT=wq,
                                         rhs=x_bf[:, c * D + S:(c + 1) * D],
                                         start=(c == 0),
                                         stop=(c == NC_CHUNK - 1),
                                         skip_group_check=True)
                    cl_sb = sm.tile([5, D], f32, tag="clsb")
                    nc.scalar.copy(cl_sb[:, 0:S], cl_a)
                    nc.scalar.copy(cl_sb[:, S:D], cl_b)
                    nc.scalar.dma_start(out=out[b, N_HEAD_OUT:256, :], in_=cl_sb)

                return [p1, p2, p3, p4, p5]

            # ======== pipeline: stage A(b) || finalize pieces of b-1 ========
            pending = []
            for b in range(EX):
                # x rows for the cluster matmul of example b, loaded mid-stream
                x_t = xp.tile([P, NC_CHUNK * D], f32, tag="x")

                def x_load(b=b, x_t=x_t):
                    (nc.sync if b % 2 == 0 else nc.scalar).dma_start(
                        out=x_t.rearrange("p (k d) -> p k d", k=NC_CHUNK),
                        in_=x_in[b * S:(b + 1) * S, :].rearrange(
                            "(k p) d -> p k d", p=P))

                if b > 0:
                    emit_diag_dma(b - 1)
                hi_ps, lo_ps = emit_stage_a(b, pending, x_load)
                pending = finalize_pieces(b, hi_ps, lo_ps, x_t)
            emit_diag_dma(EX - 1)
            for pc in pending:
                pc()

    nc.compile()
    return nc


_NC_CACHE = {}


def _consts():
    tri = np.zeros((P, NC_CHUNK * S), np.float16)
    for c in range(NC_CHUNK):
        for p in range(P):
            tri[p, c * S + c * P + p:(c + 1) * S] = 1.0
    iota2 = (np.arange(P, dtype=np.float32)[:, None]
             + np.array([0.0, 128.0], np.float32)[None, :])
    lowb = np.tile((53.0 * np.arange(5, dtype=np.float32) + 0.5)[None, :], (P, 1))
    highb = np.tile((53.0 * np.arange(5, dtype=np.float32) + 53.5)[None, :], (P, 1))
    return {
        "c_ones_p_f16": np.ones((P, 1), np.float16),
        "c_ones_p_f32": np.ones((P, 1), np.float32),
        "c_ones_r_f32": np.ones((1, P), np.float32),
        "c_ones_r_f16": np.ones((1, P), np.float16),
        "c_triu": np.triu(np.ones((P, P))).astype(np.float16),
        "c_ones_sq": np.ones((P, P), np.float16),
        "c_tri_inc": tri,
        "c_iota2": iota2,
        "c_ones_1": np.ones((1, 1), np.float32),
        "c_lowb": lowb,
        "c_highb": highb,
    }


def kernel(x: np.ndarray, atten: np.ndarray, trace: bool = False):
    if "nc" not in _NC_CACHE:
        _NC_CACHE["nc"] = build_nc()
    nc = _NC_CACHE["nc"]
    x = np.ascontiguousarray(np.asarray(x, np.float32))
    atten = np.ascontiguousarray(np.asarray(atten, np.float32))
    consts = _consts()
    in_maps = []
    for ci in range(N_CORES):
        in_maps.append({
            "x": x[ci * EX:(ci + 1) * EX].reshape(EX * S, D),
            "atten": atten[ci * EX * H:(ci + 1) * EX * H],
            **consts,
        })
    res = run_bass_kernel_spmd(nc, in_maps, list(range(N_CORES)), trace=trace)
    _NC_CACHE["last_res"] = res
    out = np.concatenate([res.results[ci]["out"] for ci in range(N_CORES)], axis=0)
    if trace:
        return out, res
    return out
